# revision 32
# baseline (speedup 1.0000x reference)
"""CPCA-weighted loss kernel for 8 Trainium2 NeuronCores.

Sharding: data-parallel over the env dim n (8 envs -> 1 env/core, params
replicated).  Each core runs the k=16-step GRU over its 256 sequences, the
two-layer classifier for pos/neg logits, softplus + weighted-mask reduce,
and returns per-core partial sums; the host combines them into the scalar
loss.

Restructurings vs the reference (exact math, different schedule):
  * x @ W_ih.T has only 8 distinct rows (7 actions + zero pad) ->
    precompute G8 = [emb;0] @ W_ih.T + b_ih on the host and fold the r/z
    part into the gate matmul as a K=8 augmented accumulation against a
    one-hot rhs (which also adds b_hh via the column-sum-1 property).
    The n-gate part is added after r*gh_n as a column slice of a
    precomputed (512, 271) table.
  * act_seq / pos_exp / neg_exp "unfold" gathers are shift-by-m column
    slices of padded tensors -- no gather on device.
  * classify(concat([e, q], -1)) = e @ W1a.T + q @ W1b.T (+b1); the
    e-part is computed once per core from vision/negatives, and indexed
    by column slices.
  * logits are built transposed (batch on partitions, step m on the free
    dim) so softplus + mask-weight-reduce run on 128 lanes.
"""

import os

import numpy as np
import ml_dtypes

import concourse.bass as bass
import concourse.mybir as mybir
import concourse.tile as tile
from concourse import bacc
from concourse.bass_utils import run_bass_kernel_spmd

BF16 = mybir.dt.bfloat16
F32 = mybir.dt.float32
AF = mybir.ActivationFunctionType
ALU = mybir.AluOpType

T, N, H, K, A = 256, 8, 512, 16, 4
NUM_ACTIONS = 6
P_SUB = 0.1
LOSS_FACTOR = 0.1
WEIGHT = np.array([5, 4, 3, 3, 2, 2, 2, 2, 1, 1, 1, 1, 1, 1, 1, 1], dtype=np.float32)

NCORES = 8
B = T * N // NCORES            # 256 sequences per core
BC = B // 128                  # 2 partition chunks of the batch
HC = H // 128                  # 4 partition chunks of the hidden dim
G = 3 * H                      # 1536 gate dim
GC = G // 128                  # 12 gate chunks (0-3 r, 4-7 z, 8-11 n)
PADW = T + K - 1               # 271 padded action-sequence length

_NC_CACHE = {}


def _build_bass():
    """Build the per-core Bass program (identical on all 8 cores)."""
    if "nc" in _NC_CACHE:
        return _NC_CACHE["nc"]
    # debugging aid: KPHASE in {gru, cls, sp, full} truncates the program
    phase = os.environ.get("KPHASE", "full")

    nc = bacc.Bacc("TRN2", target_bir_lowering=False, debug=False)

    # --- DRAM I/O ------------------------------------------------------
    d_wt = nc.dram_tensor("wt", [128, HC, G], BF16, kind="ExternalInput")
    d_gaug = nc.dram_tensor("gaug", [96, G], BF16, kind="ExternalInput")
    d_onehot = nc.dram_tensor("onehot", [96, PADW], BF16, kind="ExternalInput")
    d_gin = nc.dram_tensor("gin", [128, HC, PADW], BF16, kind="ExternalInput")
    d_belief = nc.dram_tensor("belief", [128, HC, B], BF16, kind="ExternalInput")
    d_vis = nc.dram_tensor("vis", [128, HC, B], BF16, kind="ExternalInput")
    d_neg = nc.dram_tensor("neg", [128, HC, B], BF16, kind="ExternalInput")
    d_w1 = nc.dram_tensor("w1abt", [128, 8, 32], BF16, kind="ExternalInput")
    d_w2 = nc.dram_tensor("w2t", [32, 1], BF16, kind="ExternalInput")
    d_b1 = nc.dram_tensor("b1", [32, 1], F32, kind="ExternalInput")
    d_b2n = nc.dram_tensor("b2neg", [128, 1], F32, kind="ExternalInput")
    d_wmp = nc.dram_tensor("wmaskp", [128, BC, K], F32, kind="ExternalInput")
    d_wmn = nc.dram_tensor("wmaskn", [128, BC, K], F32, kind="ExternalInput")
    d_out = nc.dram_tensor("partials", [128, 4], F32, kind="ExternalOutput")

    with tile.TileContext(nc) as tc:
        with (
            tc.tile_pool(name="const", bufs=1) as const,
            tc.tile_pool(name="hpool", bufs=1) as hpool,
            tc.tile_pool(name="gates", bufs=1) as gates,
            tc.tile_pool(name="cls", bufs=1) as cls,
            tc.tile_pool(name="gpsum", bufs=1, space="PSUM") as gpsum,
            tc.tile_pool(name="spsum", bufs=1, space="PSUM") as spsum,
            tc.tile_pool(name="lpsum", bufs=1, space="PSUM") as lpsum,
        ):
            # --- constants into SBUF (weights first, split for queue
            # parallelism; late-needed tensors last) ---------------------
            # spread the startup DMAs over several engine queues
            wt = const.tile([128, HC, G], BF16)
            dma_engines = [nc.sync, nc.gpsimd, nc.scalar, nc.sync]
            for k in range(HC):
                dma_engines[k].dma_start(wt[:, k, :], d_wt[:, k, :])
            gaug = const.tile([96, G], BF16)
            nc.sync.dma_start(gaug[:], d_gaug[:])
            onehot = const.tile([96, PADW], BF16)
            nc.gpsimd.dma_start(onehot[:], d_onehot[:])
            # h0 = belief, one tile per hidden chunk
            h = []
            for k in range(HC):
                ht = hpool.tile([128, B], BF16, tag="h", bufs=12, name=f"h0_{k}")
                dma_engines[k].dma_start(ht[:], d_belief[:, k, :])
                h.append(ht)
            gin = const.tile([128, HC, PADW], BF16)
            nc.gpsimd.dma_start(gin[:], d_gin[:])
            w1 = const.tile([128, 8, 32], BF16)
            nc.scalar.dma_start(w1[:], d_w1[:])
            vis = const.tile([128, HC, B], BF16)
            nc.sync.dma_start(vis[:], d_vis[:])
            neg = const.tile([128, HC, B], BF16)
            nc.sync.dma_start(neg[:], d_neg[:])
            w2 = const.tile([32, 1], BF16)
            nc.sync.dma_start(w2[:], d_w2[:])
            b1 = const.tile([32, 1], F32)
            nc.sync.dma_start(b1[:], d_b1[:])
            b2n = const.tile([128, 1], F32)
            nc.sync.dma_start(b2n[:], d_b2n[:])
            wmp = const.tile([128, BC, K], F32)
            nc.sync.dma_start(wmp[:], d_wmp[:])
            wmn = const.tile([128, BC, K], F32)
            nc.sync.dma_start(wmn[:], d_wmn[:])

            # --- V1padT / N1padT: e-part of the classifier ------------
            # V1padT[:, c] = (vision[c] @ W1a.T).T for c<256, 0 for pads
            vpads = []
            for nm, src in (("v1", vis), ("n1", neg)):
                ps = spsum.tile([32, B], F32, tag="s", bufs=2, name=f"ps_{nm}")
                for k in range(HC):
                    nc.tensor.matmul(
                        ps[:], w1[:, k, :], src[:, k, :],
                        start=(k == 0), stop=(k == HC - 1),
                    )
                vp = cls.tile([32, PADW + 1], BF16, tag=f"{nm}pad", name=f"{nm}pad")
                nc.vector.memset(vp[:, T:], 0.0)
                nc.scalar.activation(vp[:, 0:T], ps[:], AF.Copy)
                vpads.append(vp)
            v1pad, n1pad = vpads

            # logit staging: [batch-part, (pos0,pos1,neg0,neg1), m]
            logits = cls.tile([128, 4, K], F32, tag="logits", name="logits")

            # --- GRU + classifier loop --------------------------------
            for m in range(K):
                # Per hidden-chunk j: r/z/n gate matmuls then the full gate
                # chain, so h_new[j] is ready while the PE still works on
                # later chunks (keeps the PE dense across step boundaries
                # and HAM at K=8/8).  The onehot-augmented matmul goes
                # first in each accumulation group: it does not depend on
                # h, so next-step PE work is issueable during the gate
                # latency window.
                hn = []
                for j in range(HC):
                    # onehot-augmented matmuls first (h-independent; K=8 on
                    # row strips 0/32/64 -> the three run concurrently)
                    pss = {}
                    for s, part in enumerate((j, HC + j, 2 * HC + j)):
                        ps = gpsum.tile([128, B], F32, tag="g", bufs=5,
                                        name=f"g{m}_{part}")
                        nc.tensor.matmul(
                            ps[:], gaug[32 * s:32 * s + 8, bass.ts(part, 128)],
                            onehot[32 * s:32 * s + 8, m:m + B],
                            start=True, stop=False, tile_position=(32 * s, 0),
                        )
                        pss[part] = ps
                    for part in (j, HC + j, 2 * HC + j):
                        ps = pss[part]
                        for k in range(HC):
                            nc.tensor.matmul(
                                ps[:], wt[:, k, bass.ts(part, 128)], h[k][:],
                                start=False, stop=(k == HC - 1),
                            )
                    r = gates.tile([128, B], BF16, tag="r", bufs=6, name=f"r{m}_{j}")
                    nc.scalar.activation(r[:], pss[j][:], AF.Sigmoid)
                    z = gates.tile([128, B], BF16, tag="z", bufs=6, name=f"z{m}_{j}")
                    nc.scalar.activation(z[:], pss[HC + j][:], AF.Sigmoid)
                    tmp = gates.tile([128, B], BF16, tag="tmp", bufs=6, name=f"t{m}_{j}")
                    nc.vector.tensor_mul(tmp[:], r[:], pss[2 * HC + j][:])
                    tmp2 = gates.tile([128, B], BF16, tag="tmp2", bufs=6, name=f"u{m}_{j}")
                    nc.vector.tensor_add(tmp2[:], tmp[:], gin[:, j, m:m + B])
                    cand = gates.tile([128, B], BF16, tag="cand", bufs=6, name=f"c{m}_{j}")
                    nc.scalar.activation(cand[:], tmp2[:], AF.Tanh)
                    # h' = cand + z*(h-cand); chunks 2,3 on the otherwise
                    # idle GpSimd engine to unload the DVE
                    ve = nc.vector if j < 2 else nc.gpsimd
                    d = gates.tile([128, B], BF16, tag="d", bufs=6, name=f"d{m}_{j}")
                    ve.tensor_sub(d[:], h[j][:], cand[:])
                    e = gates.tile([128, B], BF16, tag="e", bufs=6, name=f"e{m}_{j}")
                    ve.tensor_mul(e[:], z[:], d[:])
                    ht = hpool.tile([128, B], BF16, tag="h", bufs=12, name=f"h{m}_{j}")
                    ve.tensor_add(ht[:], cand[:], e[:])
                    hn.append(ht)
                h = hn
                if phase == "gru":
                    continue

                # classifier for step m: q-part then pos/neg heads
                qp = spsum.tile([32, B], F32, tag="s", bufs=2, name=f"q{m}")
                for k in range(HC):
                    nc.tensor.matmul(
                        qp[:], w1[:, HC + k, :], h[k][:],
                        start=(k == 0), stop=(k == HC - 1),
                    )
                pl = lpsum.tile([128, 4], F32, tag="l", bufs=1, name=f"pl{m}")
                for c, vp in ((0, v1pad), (1, n1pad)):
                    pre = cls.tile([32, B], BF16, tag="pre", bufs=4, name=f"pre{m}_{c}")
                    nc.vector.scalar_tensor_tensor(
                        out=pre[:], in0=qp[:], scalar=b1[:, 0:1],
                        in1=vp[:, m + 1:m + 1 + B],
                        op0=ALU.add, op1=ALU.add,
                    )
                    h1 = cls.tile([32, B], BF16, tag="h1", bufs=4, name=f"h1{m}_{c}")
                    nc.vector.tensor_scalar_max(h1[:], pre[:], 0.0)
                    for ch in range(BC):
                        nc.tensor.matmul(
                            pl[:, 2 * c + ch:2 * c + ch + 1],
                            h1[:, bass.ts(ch, 128)], w2[:],
                            start=True, stop=True,
                        )
                nc.scalar.activation(logits[:, :, m], pl[:], AF.Copy)

            # --- softplus + mask-weight reduce ------------------------
            # softplus(s) = max(s,0) + ln(1 + exp(-|s|)), s = -+(logit + b2)
            partials = cls.tile([128, 4], F32, tag="part", name="partials")
            if phase == "gru":
                nc.scalar.activation(partials[:], h[0][:, 0:4], AF.Copy)
            elif phase == "cls":
                nc.scalar.activation(partials[:], logits[:, :, 0], AF.Copy)
            else:
                for c in range(4):
                    pos = c < 2
                    ch = c % 2
                    s = cls.tile([128, K], F32, tag="s_aff", bufs=2, name=f"s{c}")
                    nc.vector.tensor_scalar(
                        out=s[:], in0=logits[:, c, :],
                        scalar1=b2n[:, 0:1], scalar2=-1.0 if pos else 1.0,
                        op0=ALU.add, op1=ALU.mult,
                    )
                    rl = cls.tile([128, K], F32, tag="s_rl", bufs=2, name=f"rl{c}")
                    nc.vector.tensor_scalar_max(rl[:], s[:], 0.0)
                    nab = cls.tile([128, K], F32, tag="s_nab", bufs=2, name=f"na{c}")
                    # -|s| = s - 2*max(s,0)
                    nc.vector.scalar_tensor_tensor(
                        out=nab[:], in0=rl[:], scalar=-2.0, in1=s[:],
                        op0=ALU.mult, op1=ALU.add,
                    )
                    ex = cls.tile([128, K], F32, tag="s_ex", bufs=2, name=f"ex{c}")
                    nc.scalar.activation(ex[:], nab[:], AF.Exp)
                    lg = cls.tile([128, K], F32, tag="s_lg", bufs=2, name=f"lg{c}")
                    nc.scalar.activation(lg[:], ex[:], AF.Ln, bias=1.0)
                    sp = cls.tile([128, K], F32, tag="sp", bufs=2, name=f"sp{c}")
                    nc.vector.tensor_add(sp[:], rl[:], lg[:])
                    # (tensor_tensor_reduce crashes this runtime; use mul+reduce)
                    tr = cls.tile([128, K], F32, tag="tr", bufs=2, name=f"tr{c}")
                    wm = wmp if pos else wmn
                    nc.vector.tensor_mul(tr[:], sp[:], wm[:, ch, :])
                    nc.vector.reduce_sum(partials[:, c:c + 1], tr[:],
                                         axis=mybir.AxisListType.X)
            nc.sync.dma_start(d_out[:], partials[:])

    nc.compile()
    _NC_CACHE["nc"] = nc
    return nc


def _threefry_pair(k0, k1, x0, x1):
    """numpy port of jax's threefry2x32 primitive (verified bit-exact)."""
    x0 = x0.astype(np.uint32).copy()
    x1 = x1.astype(np.uint32).copy()
    ks0 = np.uint32(k0)
    ks1 = np.uint32(k1)
    ks2 = np.uint32(ks0 ^ ks1 ^ np.uint32(0x1BD11BDA))

    def rotl(x, d):
        return ((x << np.uint32(d)) | (x >> np.uint32(32 - d))).astype(np.uint32)

    rots = [[13, 15, 26, 6], [17, 29, 16, 24]]
    x0 = (x0 + ks0).astype(np.uint32)
    x1 = (x1 + ks1).astype(np.uint32)
    ks = [ks1, ks2, ks0]
    for i in range(5):
        for r in rots[i % 2]:
            x0 = (x0 + x1).astype(np.uint32)
            x1 = np.uint32(rotl(x1, r) ^ x0)
        x0 = (x0 + ks[i % 3]).astype(np.uint32)
        x1 = (x1 + ks[(i + 1) % 3] + np.uint32(i + 1)).astype(np.uint32)
    return x0, x1


def _uniform_lt(key, shape, thresh):
    """jax.random.uniform(key, shape) < thresh, threefry-partitionable spec."""
    num = int(np.prod(shape))
    b1, b2 = _threefry_pair(key[0], key[1], np.zeros(num, np.uint32),
                            np.arange(num, dtype=np.uint32))
    bits = b1 ^ b2
    fl = ((bits >> np.uint32(9)) | np.uint32(0x3F800000)).view(np.float32) \
        - np.float32(1.0)
    fl = np.maximum(fl, np.float32(0.0))
    return (fl < np.float32(thresh)).reshape(shape)


def _sub_masks():
    """The reference's input-independent Bernoulli(P_SUB) masks
    (jax.random key(42) -> split -> uniform < P_SUB)."""
    if "subs" not in _NC_CACHE:
        b1, b2 = _threefry_pair(0, 42, np.zeros(2, np.uint32),
                                np.arange(2, dtype=np.uint32))
        sub_p = _uniform_lt((b1[0], b2[0]), (T, K, N), P_SUB)
        sub_n = _uniform_lt((b1[1], b2[1]), (T, K, N), P_SUB)
        _NC_CACHE["subs"] = (sub_p, sub_n)
    return _NC_CACHE["subs"]


def _bf16(x):
    return np.ascontiguousarray(np.asarray(x, dtype=np.float32)).astype(
        ml_dtypes.bfloat16
    )


def build_in_maps(inputs):
    """Host-side prep: returns (in_maps, cnt_p, cnt_n)."""
    return _prep(**{k: v for k, v in inputs.items() if k not in ("t", "n")})


def _prep(vision, belief_features, actions, env_zeros, negative_inds,
          emb, W_ih, W_hh, b_ih, b_hh, W1, b1, W2, b2, **_unused):
    vision = np.asarray(vision, np.float32)
    belief = np.asarray(belief_features, np.float32)
    actions = np.asarray(actions, np.int64)
    env_zeros = np.asarray(env_zeros, np.int64)
    negative_inds = np.asarray(negative_inds, np.int64)
    emb = np.asarray(emb, np.float32)
    W_ih = np.asarray(W_ih, np.float32)
    W_hh = np.asarray(W_hh, np.float32)
    b_ih = np.asarray(b_ih, np.float32)
    b_hh = np.asarray(b_hh, np.float32)
    W1 = np.asarray(W1, np.float32)
    b1v = np.asarray(b1, np.float32)
    W2 = np.asarray(W2, np.float32)
    b2v = np.asarray(b2, np.float32)

    # ---- host-side parameter folding (O(params) only) -----------------
    # G8[a] = x_a @ W_ih.T + b_ih for the 7 actions + zero pad (row 7)
    G8 = np.concatenate([emb, np.zeros((1, A), np.float32)], 0) @ W_ih.T + b_ih
    gaug8 = np.empty((8, G), np.float32)
    gaug8[:, :2 * H] = G8[:, :2 * H] + b_hh[None, :2 * H]    # r,z: gi + b_hh
    gaug8[:, 2 * H:] = np.tile(b_hh[None, 2 * H:], (8, 1))   # n:   b_hh only
    # replicate at partition strips 0/32/64 for row-tiled concurrent matmuls
    gaug = np.zeros((96, G), np.float32)
    for s in range(3):
        gaug[32 * s:32 * s + 8] = gaug8
    wt = np.ascontiguousarray(
        W_hh.T.reshape(HC, 128, G).transpose(1, 0, 2))       # [128, HC, G]
    w1abt = np.ascontiguousarray(
        W1.T.reshape(8, 128, 32).transpose(1, 0, 2))         # [128, 8, 32]
    w2t = W2.reshape(1, 32).T                                # [32, 1]

    # ---- masks (host): valid & subsample, weighted --------------------
    sub_p, sub_n = _sub_masks()
    r = np.arange(T + K)[:, None, None]
    c = np.arange(K)[None, :, None]
    z = env_zeros[None, None, :, :]
    zero_hit = np.any((z >= (r - c + 1)[..., None]) & (z <= (r + 1)[..., None]),
                      axis=-1)
    valid_full = (r >= c) & (r < T - 1) & (~zero_hit)        # (T+K, K, N)
    idx = np.arange(T)[:, None] + np.arange(K)[None, :]
    valid = valid_full[idx, np.arange(K)[None, :]]           # (T, K, N)
    mask_p = valid & sub_p
    mask_n = valid & sub_n
    wmask_p = WEIGHT[None, :, None] * mask_p                 # (T, K, N) f32
    wmask_n = WEIGHT[None, :, None] * mask_n
    cnt_p = float(mask_p.sum())
    cnt_n = float(mask_n.sum())

    # ---- per-core inputs ----------------------------------------------
    negatives = vision.reshape(T * N, H)[negative_inds].reshape(T, N, H)

    def chunkT(x):  # (T=B, H) -> [128, HC, B] feature-major chunks
        return np.ascontiguousarray(x.T.reshape(HC, 128, B).transpose(1, 0, 2))

    in_maps = []
    for e in range(NCORES):
        a_pad = np.concatenate([actions[:, e], np.full(K - 1, 7, np.int64)])
        onehot = np.zeros((96, PADW), np.float32)
        for s in range(3):
            onehot[32 * s + a_pad, np.arange(PADW)] = 1.0
        gin = np.ascontiguousarray(
            G8[a_pad][:, 2 * H:].T.reshape(HC, 128, PADW).transpose(1, 0, 2))
        in_maps.append({
            "wt": _bf16(wt),
            "gaug": _bf16(gaug),
            "onehot": _bf16(onehot),
            "gin": _bf16(gin),
            "belief": _bf16(chunkT(belief[:, e, :])),
            "vis": _bf16(chunkT(vision[:, e, :])),
            "neg": _bf16(chunkT(negatives[:, e, :])),
            "w1abt": _bf16(w1abt),
            "w2t": _bf16(w2t),
            "b1": np.ascontiguousarray(b1v.reshape(32, 1)),
            "b2neg": np.full((128, 1), float(b2v.reshape(-1)[0]), np.float32),
            "wmaskp": np.ascontiguousarray(
                wmask_p[:, :, e].reshape(BC, 128, K).transpose(1, 0, 2)),
            "wmaskn": np.ascontiguousarray(
                wmask_n[:, :, e].reshape(BC, 128, K).transpose(1, 0, 2)),
        })

    return in_maps, cnt_p, cnt_n


def kernel(**inputs):
    in_maps, cnt_p, cnt_n = build_in_maps(inputs)
    nc = _build_bass()
    res = run_bass_kernel_spmd(nc, in_maps, core_ids=list(range(NCORES)))
    parts = np.stack([res.results[i]["partials"] for i in range(NCORES)])
    sp_num = float(parts[:, :, 0:2].sum(dtype=np.float64))
    sn_num = float(parts[:, :, 2:4].sum(dtype=np.float64))
    loss = (sp_num / max(cnt_p, 1.0) + sn_num / max(cnt_n, 1.0)) * LOSS_FACTOR
    return np.float32(loss)


# revision 33
# speedup vs baseline: 1.3125x; 1.3125x over previous
"""CPCA-weighted loss kernel for 8 Trainium2 NeuronCores.

Sharding: data-parallel over the env dim n (8 envs -> 1 env/core, params
replicated).  Each core runs the k=16-step GRU over its 256 sequences, the
two-layer classifier for pos/neg logits, softplus + weighted-mask reduce,
and returns per-core partial sums; the host combines them into the scalar
loss.

Restructurings vs the reference (exact math, different schedule):
  * x @ W_ih.T has only 8 distinct rows (7 actions + zero pad) ->
    precompute G8 = [emb;0] @ W_ih.T + b_ih on the host and fold the r/z
    part into the gate matmul as a K=8 augmented accumulation against a
    one-hot rhs (which also adds b_hh via the column-sum-1 property).
    The n-gate part is added after r*gh_n as a column slice of a
    precomputed (512, 271) table.
  * act_seq / pos_exp / neg_exp "unfold" gathers are shift-by-m column
    slices of padded tensors -- no gather on device.
  * classify(concat([e, q], -1)) = e @ W1a.T + q @ W1b.T (+b1); the
    e-part is computed once per core from vision/negatives, and indexed
    by column slices.
  * logits are built transposed (batch on partitions, step m on the free
    dim) so softplus + mask-weight-reduce run on 128 lanes.
"""

import os

import numpy as np
import ml_dtypes

import concourse.bass as bass
import concourse.mybir as mybir
import concourse.tile as tile
from concourse import bacc
from concourse.bass_utils import run_bass_kernel_spmd

BF16 = mybir.dt.bfloat16
F32 = mybir.dt.float32
AF = mybir.ActivationFunctionType
ALU = mybir.AluOpType

T, N, H, K, A = 256, 8, 512, 16, 4
NUM_ACTIONS = 6
P_SUB = 0.1
LOSS_FACTOR = 0.1
WEIGHT = np.array([5, 4, 3, 3, 2, 2, 2, 2, 1, 1, 1, 1, 1, 1, 1, 1], dtype=np.float32)

NCORES = 8
B = T * N // NCORES            # 256 sequences per core
BC = B // 128                  # 2 partition chunks of the batch
HC = H // 128                  # 4 partition chunks of the hidden dim
G = 3 * H                      # 1536 gate dim
GC = G // 128                  # 12 gate chunks (0-3 r, 4-7 z, 8-11 n)
PADW = T + K - 1               # 271 padded action-sequence length

_NC_CACHE = {}


def _build_bass():
    """Build the per-core Bass program (identical on all 8 cores)."""
    if "nc" in _NC_CACHE:
        return _NC_CACHE["nc"]
    # debugging aid: KPHASE in {gru, cls, sp, full} truncates the program
    phase = os.environ.get("KPHASE", "full")

    nc = bacc.Bacc("TRN2", target_bir_lowering=False, debug=False)

    # --- DRAM I/O ------------------------------------------------------
    d_wt = nc.dram_tensor("wt", [128, HC, G], BF16, kind="ExternalInput")
    d_gaug = nc.dram_tensor("gaug", [96, G], BF16, kind="ExternalInput")
    d_onehot = nc.dram_tensor("onehot", [96, PADW], BF16, kind="ExternalInput")
    d_gin = nc.dram_tensor("gin", [128, HC, PADW], BF16, kind="ExternalInput")
    d_belief = nc.dram_tensor("belief", [128, HC, B], BF16, kind="ExternalInput")
    d_vis = nc.dram_tensor("vis", [128, HC, B], BF16, kind="ExternalInput")
    d_neg = nc.dram_tensor("neg", [128, HC, B], BF16, kind="ExternalInput")
    d_w1 = nc.dram_tensor("w1abt", [128, 8, 32], BF16, kind="ExternalInput")
    d_w2 = nc.dram_tensor("w2t", [32, 1], BF16, kind="ExternalInput")
    d_b1 = nc.dram_tensor("b1", [32, 1], F32, kind="ExternalInput")
    d_b2n = nc.dram_tensor("b2neg", [128, 1], F32, kind="ExternalInput")
    d_wmp = nc.dram_tensor("wmaskp", [128, BC, K], F32, kind="ExternalInput")
    d_wmn = nc.dram_tensor("wmaskn", [128, BC, K], F32, kind="ExternalInput")
    d_out = nc.dram_tensor("partials", [128, 4], F32, kind="ExternalOutput")

    with tile.TileContext(nc) as tc:
        with (
            tc.tile_pool(name="const", bufs=1) as const,
            tc.tile_pool(name="hpool", bufs=1) as hpool,
            tc.tile_pool(name="gates", bufs=1) as gates,
            tc.tile_pool(name="cls", bufs=1) as cls,
            tc.tile_pool(name="gpsum", bufs=1, space="PSUM") as gpsum,
            tc.tile_pool(name="spsum", bufs=1, space="PSUM") as spsum,
            tc.tile_pool(name="lpsum", bufs=1, space="PSUM") as lpsum,
        ):
            # --- constants into SBUF (weights first, split for queue
            # parallelism; late-needed tensors last) ---------------------
            # spread the startup DMAs over several engine queues
            wt = const.tile([128, HC, G], BF16)
            dma_engines = [nc.sync, nc.gpsimd, nc.sync, nc.gpsimd]
            for k in range(HC):
                dma_engines[k].dma_start(wt[:, k, :], d_wt[:, k, :])
            gaug = const.tile([96, G], BF16)
            nc.sync.dma_start(gaug[:], d_gaug[:])
            onehot = const.tile([96, PADW], BF16)
            nc.gpsimd.dma_start(onehot[:], d_onehot[:])
            # h0 = belief, one tile per hidden chunk
            h = []
            for k in range(HC):
                ht = hpool.tile([128, B], BF16, tag="h", bufs=12, name=f"h0_{k}")
                dma_engines[k].dma_start(ht[:], d_belief[:, k, :])
                h.append(ht)
            gin = const.tile([128, HC, PADW], BF16)
            nc.gpsimd.dma_start(gin[:], d_gin[:])
            w1 = const.tile([128, 8, 32], BF16)
            nc.gpsimd.dma_start(w1[:], d_w1[:])
            vis = const.tile([128, HC, B], BF16)
            nc.sync.dma_start(vis[:], d_vis[:])
            neg = const.tile([128, HC, B], BF16)
            nc.sync.dma_start(neg[:], d_neg[:])
            w2 = const.tile([32, 1], BF16)
            nc.sync.dma_start(w2[:], d_w2[:])
            b1 = const.tile([32, 1], F32)
            nc.sync.dma_start(b1[:], d_b1[:])
            b2n = const.tile([128, 1], F32)
            nc.sync.dma_start(b2n[:], d_b2n[:])
            wmp = const.tile([128, BC, K], F32)
            nc.sync.dma_start(wmp[:], d_wmp[:])
            wmn = const.tile([128, BC, K], F32)
            nc.sync.dma_start(wmn[:], d_wmn[:])

            # --- V1padT / N1padT: e-part of the classifier ------------
            # V1padT[:, c] = (vision[c] @ W1a.T).T for c<256, 0 for pads
            vpads = []
            for nm, src in (("v1", vis), ("n1", neg)):
                ps = spsum.tile([32, B], F32, tag="s", bufs=2, name=f"ps_{nm}")
                for k in range(HC):
                    nc.tensor.matmul(
                        ps[:], w1[:, k, :], src[:, k, :],
                        start=(k == 0), stop=(k == HC - 1),
                    )
                vp = cls.tile([32, PADW + 1], BF16, tag=f"{nm}pad", name=f"{nm}pad")
                nc.vector.memset(vp[:, T:], 0.0)
                nc.scalar.activation(vp[:, 0:T], ps[:], AF.Copy)
                vpads.append(vp)
            v1pad, n1pad = vpads

            # logit staging: [batch-part, (pos0,pos1,neg0,neg1), m]
            logits = cls.tile([128, 4, K], F32, tag="logits", name="logits")

            # --- GRU + classifier loop --------------------------------
            for m in range(K):
                # Per hidden-chunk j: r/z/n gate matmuls then the full gate
                # chain, so h_new[j] is ready while the PE still works on
                # later chunks (keeps the PE dense across step boundaries
                # and HAM at K=8/8).  The onehot-augmented matmul goes
                # first in each accumulation group: it does not depend on
                # h, so next-step PE work is issueable during the gate
                # latency window.
                hn = []
                for j in range(HC):
                    # onehot-augmented matmuls first (h-independent; K=8 on
                    # row strips 0/32/64 -> the three run concurrently)
                    pss = {}
                    for s, part in enumerate((j, HC + j, 2 * HC + j)):
                        ps = gpsum.tile([128, B], F32, tag="g", bufs=4,
                                        name=f"g{m}_{part}")
                        nc.tensor.matmul(
                            ps[:], gaug[32 * s:32 * s + 8, bass.ts(part, 128)],
                            onehot[32 * s:32 * s + 8, m:m + B],
                            start=True, stop=False, tile_position=(32 * s, 0),
                        )
                        pss[part] = ps
                    for part in (j, HC + j, 2 * HC + j):
                        ps = pss[part]
                        for k in range(HC):
                            nc.tensor.matmul(
                                ps[:], wt[:, k, bass.ts(part, 128)], h[k][:],
                                start=False, stop=(k == HC - 1),
                            )
                    r = gates.tile([128, B], BF16, tag="r", bufs=6, name=f"r{m}_{j}")
                    nc.scalar.activation(r[:], pss[j][:], AF.Sigmoid)
                    z = gates.tile([128, B], BF16, tag="z", bufs=6, name=f"z{m}_{j}")
                    nc.scalar.activation(z[:], pss[HC + j][:], AF.Sigmoid)
                    tmp = gates.tile([128, B], BF16, tag="tmp", bufs=6, name=f"t{m}_{j}")
                    nc.vector.tensor_mul(tmp[:], r[:], pss[2 * HC + j][:])
                    tmp2 = gates.tile([128, B], BF16, tag="tmp2", bufs=6, name=f"u{m}_{j}")
                    nc.vector.tensor_add(tmp2[:], tmp[:], gin[:, j, m:m + B])
                    cand = gates.tile([128, B], BF16, tag="cand", bufs=6, name=f"c{m}_{j}")
                    nc.scalar.activation(cand[:], tmp2[:], AF.Tanh)
                    ve = nc.vector
                    d = gates.tile([128, B], BF16, tag="d", bufs=6, name=f"d{m}_{j}")
                    ve.tensor_sub(d[:], h[j][:], cand[:])
                    e = gates.tile([128, B], BF16, tag="e", bufs=6, name=f"e{m}_{j}")
                    ve.tensor_mul(e[:], z[:], d[:])
                    ht = hpool.tile([128, B], BF16, tag="h", bufs=12, name=f"h{m}_{j}")
                    ve.tensor_add(ht[:], cand[:], e[:])
                    hn.append(ht)
                h = hn
                if phase == "gru":
                    continue

                # classifier for step m: q-part then pos/neg heads
                qp = spsum.tile([32, B], F32, tag="s", bufs=2, name=f"q{m}")
                for k in range(HC):
                    nc.tensor.matmul(
                        qp[:], w1[:, HC + k, :], h[k][:],
                        start=(k == 0), stop=(k == HC - 1),
                    )
                pl = lpsum.tile([128, 4], F32, tag="l", bufs=2, name=f"pl{m}")
                for c, vp in ((0, v1pad), (1, n1pad)):
                    pre = cls.tile([32, B], BF16, tag="pre", bufs=4, name=f"pre{m}_{c}")
                    nc.vector.scalar_tensor_tensor(
                        out=pre[:], in0=qp[:], scalar=b1[:, 0:1],
                        in1=vp[:, m + 1:m + 1 + B],
                        op0=ALU.add, op1=ALU.add,
                    )
                    h1 = cls.tile([32, B], BF16, tag="h1", bufs=4, name=f"h1{m}_{c}")
                    nc.vector.tensor_scalar_max(h1[:], pre[:], 0.0)
                    for ch in range(BC):
                        nc.tensor.matmul(
                            pl[:, 2 * c + ch:2 * c + ch + 1],
                            h1[:, bass.ts(ch, 128)], w2[:],
                            start=True, stop=True,
                        )
                nc.scalar.activation(logits[:, :, m], pl[:], AF.Copy)

            # --- softplus + mask-weight reduce ------------------------
            # softplus(s) = max(s,0) + ln(1 + exp(-|s|)), s = -+(logit + b2)
            partials = cls.tile([128, 4], F32, tag="part", name="partials")
            if phase == "gru":
                nc.scalar.activation(partials[:], h[0][:, 0:4], AF.Copy)
            elif phase == "cls":
                nc.scalar.activation(partials[:], logits[:, :, 0], AF.Copy)
            else:
                for c in range(4):
                    pos = c < 2
                    ch = c % 2
                    s = cls.tile([128, K], F32, tag="s_aff", bufs=2, name=f"s{c}")
                    nc.vector.tensor_scalar(
                        out=s[:], in0=logits[:, c, :],
                        scalar1=b2n[:, 0:1], scalar2=-1.0 if pos else 1.0,
                        op0=ALU.add, op1=ALU.mult,
                    )
                    rl = cls.tile([128, K], F32, tag="s_rl", bufs=2, name=f"rl{c}")
                    nc.vector.tensor_scalar_max(rl[:], s[:], 0.0)
                    nab = cls.tile([128, K], F32, tag="s_nab", bufs=2, name=f"na{c}")
                    # -|s| = s - 2*max(s,0)
                    nc.vector.scalar_tensor_tensor(
                        out=nab[:], in0=rl[:], scalar=-2.0, in1=s[:],
                        op0=ALU.mult, op1=ALU.add,
                    )
                    ex = cls.tile([128, K], F32, tag="s_ex", bufs=2, name=f"ex{c}")
                    nc.scalar.activation(ex[:], nab[:], AF.Exp)
                    lg = cls.tile([128, K], F32, tag="s_lg", bufs=2, name=f"lg{c}")
                    nc.scalar.activation(lg[:], ex[:], AF.Ln, bias=1.0)
                    sp = cls.tile([128, K], F32, tag="sp", bufs=2, name=f"sp{c}")
                    nc.vector.tensor_add(sp[:], rl[:], lg[:])
                    # (tensor_tensor_reduce crashes this runtime; use mul+reduce)
                    tr = cls.tile([128, K], F32, tag="tr", bufs=2, name=f"tr{c}")
                    wm = wmp if pos else wmn
                    nc.vector.tensor_mul(tr[:], sp[:], wm[:, ch, :])
                    nc.vector.reduce_sum(partials[:, c:c + 1], tr[:],
                                         axis=mybir.AxisListType.X)
            nc.sync.dma_start(d_out[:], partials[:])

    nc.compile()
    _NC_CACHE["nc"] = nc
    return nc


def _threefry_pair(k0, k1, x0, x1):
    """numpy port of jax's threefry2x32 primitive (verified bit-exact)."""
    x0 = x0.astype(np.uint32).copy()
    x1 = x1.astype(np.uint32).copy()
    ks0 = np.uint32(k0)
    ks1 = np.uint32(k1)
    ks2 = np.uint32(ks0 ^ ks1 ^ np.uint32(0x1BD11BDA))

    def rotl(x, d):
        return ((x << np.uint32(d)) | (x >> np.uint32(32 - d))).astype(np.uint32)

    rots = [[13, 15, 26, 6], [17, 29, 16, 24]]
    x0 = (x0 + ks0).astype(np.uint32)
    x1 = (x1 + ks1).astype(np.uint32)
    ks = [ks1, ks2, ks0]
    for i in range(5):
        for r in rots[i % 2]:
            x0 = (x0 + x1).astype(np.uint32)
            x1 = np.uint32(rotl(x1, r) ^ x0)
        x0 = (x0 + ks[i % 3]).astype(np.uint32)
        x1 = (x1 + ks[(i + 1) % 3] + np.uint32(i + 1)).astype(np.uint32)
    return x0, x1


def _uniform_lt(key, shape, thresh):
    """jax.random.uniform(key, shape) < thresh, threefry-partitionable spec."""
    num = int(np.prod(shape))
    b1, b2 = _threefry_pair(key[0], key[1], np.zeros(num, np.uint32),
                            np.arange(num, dtype=np.uint32))
    bits = b1 ^ b2
    fl = ((bits >> np.uint32(9)) | np.uint32(0x3F800000)).view(np.float32) \
        - np.float32(1.0)
    fl = np.maximum(fl, np.float32(0.0))
    return (fl < np.float32(thresh)).reshape(shape)


def _sub_masks():
    """The reference's input-independent Bernoulli(P_SUB) masks
    (jax.random key(42) -> split -> uniform < P_SUB)."""
    if "subs" not in _NC_CACHE:
        b1, b2 = _threefry_pair(0, 42, np.zeros(2, np.uint32),
                                np.arange(2, dtype=np.uint32))
        sub_p = _uniform_lt((b1[0], b2[0]), (T, K, N), P_SUB)
        sub_n = _uniform_lt((b1[1], b2[1]), (T, K, N), P_SUB)
        _NC_CACHE["subs"] = (sub_p, sub_n)
    return _NC_CACHE["subs"]


def _bf16(x):
    return np.ascontiguousarray(np.asarray(x, dtype=np.float32)).astype(
        ml_dtypes.bfloat16
    )


def build_in_maps(inputs):
    """Host-side prep: returns (in_maps, cnt_p, cnt_n)."""
    return _prep(**{k: v for k, v in inputs.items() if k not in ("t", "n")})


def _prep(vision, belief_features, actions, env_zeros, negative_inds,
          emb, W_ih, W_hh, b_ih, b_hh, W1, b1, W2, b2, **_unused):
    vision = np.asarray(vision, np.float32)
    belief = np.asarray(belief_features, np.float32)
    actions = np.asarray(actions, np.int64)
    env_zeros = np.asarray(env_zeros, np.int64)
    negative_inds = np.asarray(negative_inds, np.int64)
    emb = np.asarray(emb, np.float32)
    W_ih = np.asarray(W_ih, np.float32)
    W_hh = np.asarray(W_hh, np.float32)
    b_ih = np.asarray(b_ih, np.float32)
    b_hh = np.asarray(b_hh, np.float32)
    W1 = np.asarray(W1, np.float32)
    b1v = np.asarray(b1, np.float32)
    W2 = np.asarray(W2, np.float32)
    b2v = np.asarray(b2, np.float32)

    # ---- host-side parameter folding (O(params) only) -----------------
    # G8[a] = x_a @ W_ih.T + b_ih for the 7 actions + zero pad (row 7)
    G8 = np.concatenate([emb, np.zeros((1, A), np.float32)], 0) @ W_ih.T + b_ih
    gaug8 = np.empty((8, G), np.float32)
    gaug8[:, :2 * H] = G8[:, :2 * H] + b_hh[None, :2 * H]    # r,z: gi + b_hh
    gaug8[:, 2 * H:] = np.tile(b_hh[None, 2 * H:], (8, 1))   # n:   b_hh only
    # replicate at partition strips 0/32/64 for row-tiled concurrent matmuls
    gaug = np.zeros((96, G), np.float32)
    for s in range(3):
        gaug[32 * s:32 * s + 8] = gaug8
    wt = np.ascontiguousarray(
        W_hh.T.reshape(HC, 128, G).transpose(1, 0, 2))       # [128, HC, G]
    w1abt = np.ascontiguousarray(
        W1.T.reshape(8, 128, 32).transpose(1, 0, 2))         # [128, 8, 32]
    w2t = W2.reshape(1, 32).T                                # [32, 1]

    # ---- masks (host): valid & subsample, weighted --------------------
    sub_p, sub_n = _sub_masks()
    r = np.arange(T + K)[:, None, None]
    c = np.arange(K)[None, :, None]
    z = env_zeros[None, None, :, :]
    zero_hit = np.any((z >= (r - c + 1)[..., None]) & (z <= (r + 1)[..., None]),
                      axis=-1)
    valid_full = (r >= c) & (r < T - 1) & (~zero_hit)        # (T+K, K, N)
    idx = np.arange(T)[:, None] + np.arange(K)[None, :]
    valid = valid_full[idx, np.arange(K)[None, :]]           # (T, K, N)
    mask_p = valid & sub_p
    mask_n = valid & sub_n
    wmask_p = WEIGHT[None, :, None] * mask_p                 # (T, K, N) f32
    wmask_n = WEIGHT[None, :, None] * mask_n
    cnt_p = float(mask_p.sum())
    cnt_n = float(mask_n.sum())

    # ---- per-core inputs ----------------------------------------------
    negatives = vision.reshape(T * N, H)[negative_inds].reshape(T, N, H)

    def chunkT(x):  # (T=B, H) -> [128, HC, B] feature-major chunks
        return np.ascontiguousarray(x.T.reshape(HC, 128, B).transpose(1, 0, 2))

    in_maps = []
    for e in range(NCORES):
        a_pad = np.concatenate([actions[:, e], np.full(K - 1, 7, np.int64)])
        onehot = np.zeros((96, PADW), np.float32)
        for s in range(3):
            onehot[32 * s + a_pad, np.arange(PADW)] = 1.0
        gin = np.ascontiguousarray(
            G8[a_pad][:, 2 * H:].T.reshape(HC, 128, PADW).transpose(1, 0, 2))
        in_maps.append({
            "wt": _bf16(wt),
            "gaug": _bf16(gaug),
            "onehot": _bf16(onehot),
            "gin": _bf16(gin),
            "belief": _bf16(chunkT(belief[:, e, :])),
            "vis": _bf16(chunkT(vision[:, e, :])),
            "neg": _bf16(chunkT(negatives[:, e, :])),
            "w1abt": _bf16(w1abt),
            "w2t": _bf16(w2t),
            "b1": np.ascontiguousarray(b1v.reshape(32, 1)),
            "b2neg": np.full((128, 1), float(b2v.reshape(-1)[0]), np.float32),
            "wmaskp": np.ascontiguousarray(
                wmask_p[:, :, e].reshape(BC, 128, K).transpose(1, 0, 2)),
            "wmaskn": np.ascontiguousarray(
                wmask_n[:, :, e].reshape(BC, 128, K).transpose(1, 0, 2)),
        })

    return in_maps, cnt_p, cnt_n


def kernel(**inputs):
    in_maps, cnt_p, cnt_n = build_in_maps(inputs)
    nc = _build_bass()
    res = run_bass_kernel_spmd(nc, in_maps, core_ids=list(range(NCORES)))
    parts = np.stack([res.results[i]["partials"] for i in range(NCORES)])
    sp_num = float(parts[:, :, 0:2].sum(dtype=np.float64))
    sn_num = float(parts[:, :, 2:4].sum(dtype=np.float64))
    loss = (sp_num / max(cnt_p, 1.0) + sn_num / max(cnt_n, 1.0)) * LOSS_FACTOR
    return np.float32(loss)


# revision 42
# speedup vs baseline: 1.4252x; 1.0859x over previous
"""CPCA-weighted loss kernel for 8 Trainium2 NeuronCores.

Sharding: data-parallel over the env dim n (8 envs -> 1 env/core, params
replicated).  Each core runs the k=16-step GRU over its 256 sequences, the
two-layer classifier for pos/neg logits, softplus + weighted-mask reduce,
and returns per-core partial sums; the host combines them into the scalar
loss.

Restructurings vs the reference (exact math, different schedule):
  * x @ W_ih.T has only 8 distinct rows (7 actions + zero pad) ->
    precompute G8 = [emb;0] @ W_ih.T + b_ih on the host and fold the r/z
    part into the gate matmul as a K=8 augmented accumulation against a
    one-hot rhs (which also adds b_hh via the column-sum-1 property).
    The n-gate part is added after r*gh_n as a column slice of a
    precomputed (512, 271) table.
  * act_seq / pos_exp / neg_exp "unfold" gathers are shift-by-m column
    slices of padded tensors -- no gather on device.
  * classify(concat([e, q], -1)) = e @ W1a.T + q @ W1b.T (+b1); the
    e-part is computed once per core from vision/negatives, and indexed
    by column slices.
  * logits are built transposed (batch on partitions, step m on the free
    dim) so softplus + mask-weight-reduce run on 128 lanes.
"""

import os

import numpy as np
import ml_dtypes

import concourse.bass as bass
import concourse.mybir as mybir
import concourse.tile as tile
from concourse import bacc
from concourse.bass_utils import run_bass_kernel_spmd

BF16 = mybir.dt.bfloat16
F32 = mybir.dt.float32
AF = mybir.ActivationFunctionType
ALU = mybir.AluOpType

T, N, H, K, A = 256, 8, 512, 16, 4
NUM_ACTIONS = 6
P_SUB = 0.1
LOSS_FACTOR = 0.1
WEIGHT = np.array([5, 4, 3, 3, 2, 2, 2, 2, 1, 1, 1, 1, 1, 1, 1, 1], dtype=np.float32)

NCORES = 8
B = T * N // NCORES            # 256 sequences per core
BC = B // 128                  # 2 partition chunks of the batch
HC = H // 128                  # 4 partition chunks of the hidden dim
G = 3 * H                      # 1536 gate dim
GC = G // 128                  # 12 gate chunks (0-3 r, 4-7 z, 8-11 n)
PADW = T + K - 1               # 271 padded action-sequence length

_NC_CACHE = {}


def _build_bass():
    """Build the per-core Bass program (identical on all 8 cores)."""
    if "nc" in _NC_CACHE:
        return _NC_CACHE["nc"]
    # debugging aid: KPHASE in {gru, cls, sp, full} truncates the program
    phase = os.environ.get("KPHASE", "full")

    nc = bacc.Bacc("TRN2", target_bir_lowering=False, debug=False)

    # --- DRAM I/O ------------------------------------------------------
    d_wt = nc.dram_tensor("wt", [128, HC, G], BF16, kind="ExternalInput")
    d_gaug = nc.dram_tensor("gaug", [96, G], BF16, kind="ExternalInput")
    d_onehot = nc.dram_tensor("onehot", [96, PADW], BF16, kind="ExternalInput")
    d_gin = nc.dram_tensor("gin", [128, HC, PADW], BF16, kind="ExternalInput")
    d_belief = nc.dram_tensor("belief", [128, HC, B], BF16, kind="ExternalInput")
    d_vis = nc.dram_tensor("vis", [128, HC, B], BF16, kind="ExternalInput")
    d_neg = nc.dram_tensor("neg", [128, HC, B], BF16, kind="ExternalInput")
    d_w1 = nc.dram_tensor("w1abt", [128, 8, 32], BF16, kind="ExternalInput")
    d_w2 = nc.dram_tensor("w2sgn", [32, 2], BF16, kind="ExternalInput")
    d_b1 = nc.dram_tensor("b1", [32, 1], F32, kind="ExternalInput")
    d_b2pat = nc.dram_tensor("b2pat", [128, 4, K], F32, kind="ExternalInput")
    d_wm = nc.dram_tensor("wmask", [128, 4, K], F32, kind="ExternalInput")
    d_out = nc.dram_tensor("partials", [128, 4], F32, kind="ExternalOutput")

    with tile.TileContext(nc) as tc:
        with (
            tc.tile_pool(name="const", bufs=1) as const,
            tc.tile_pool(name="hpool", bufs=1) as hpool,
            tc.tile_pool(name="gates", bufs=1) as gates,
            tc.tile_pool(name="cls", bufs=1) as cls,
            tc.tile_pool(name="gpsum", bufs=1, space="PSUM") as gpsum,
            tc.tile_pool(name="spsum", bufs=1, space="PSUM") as spsum,
            tc.tile_pool(name="lpsum", bufs=1, space="PSUM") as lpsum,
        ):
            # --- constants into SBUF -----------------------------------
            # DMA order tracks first-use: gate-matmul operands first (wt
            # split per gate chunk so group j only waits for its own
            # 131KB), classifier/reduce tensors last.  Two queues.
            dq = [nc.sync, nc.gpsimd]
            gaug = const.tile([96, G], BF16)
            nc.sync.dma_start(gaug[:], d_gaug[:])
            onehot = const.tile([96, PADW], BF16)
            nc.gpsimd.dma_start(onehot[:], d_onehot[:])
            h = []
            for k in range(HC):
                ht = hpool.tile([128, B], BF16, tag="h", bufs=12, name=f"h0_{k}")
                dq[k % 2].dma_start(ht[:], d_belief[:, k, :])
                h.append(ht)
            wtj = []
            for j in range(GC):
                wj = const.tile([128, HC, 128], BF16, name=f"wt{j}")
                dq[j % 2].dma_start(wj[:], d_wt[:, :, bass.ts(j, 128)])
                wtj.append(wj)
            gin = const.tile([128, HC, PADW], BF16)
            nc.gpsimd.dma_start(gin[:], d_gin[:])
            w1 = const.tile([128, 8, 32], BF16)
            nc.sync.dma_start(w1[:], d_w1[:])
            vis = const.tile([128, HC, B], BF16)
            nc.sync.dma_start(vis[:], d_vis[:])
            neg = const.tile([128, HC, B], BF16)
            nc.gpsimd.dma_start(neg[:], d_neg[:])
            w2 = const.tile([32, 2], BF16)
            nc.sync.dma_start(w2[:], d_w2[:])
            b1 = const.tile([32, 1], F32)
            nc.sync.dma_start(b1[:], d_b1[:])
            b2pat = const.tile([128, 4, K], F32)
            nc.gpsimd.dma_start(b2pat[:], d_b2pat[:])
            wm = const.tile([128, 4, K], F32)
            nc.gpsimd.dma_start(wm[:], d_wm[:])

            # --- V1padT / N1padT: e-part of the classifier ------------
            # V1padT[:, c] = (vision[c] @ W1a.T).T for c<256, 0 for pads
            vpads = []
            for nm, src in (("v1", vis), ("n1", neg)):
                ps = spsum.tile([32, B], F32, tag="s", bufs=2, name=f"ps_{nm}")
                for k in range(HC):
                    nc.tensor.matmul(
                        ps[:], w1[:, k, :], src[:, k, :],
                        start=(k == 0), stop=(k == HC - 1),
                    )
                vp = cls.tile([32, PADW + 1], BF16, tag=f"{nm}pad", name=f"{nm}pad")
                nc.vector.memset(vp[:, T:], 0.0)
                nc.scalar.activation(vp[:, 0:T], ps[:], AF.Copy)
                vpads.append(vp)
            v1pad, n1pad = vpads

            # logit staging lives in one persistent PSUM bank:
            # [batch-part, (pos0,pos1,neg0,neg1), m], sign already folded
            # (pos columns hold -(h1 @ W2.T))
            logits = lpsum.tile([128, 4, K], F32, tag="l", bufs=1,
                                name="logits")

            # --- GRU + classifier loop --------------------------------
            for m in range(K):
                # Per hidden-chunk j: r/z/n gate matmuls then the full gate
                # chain, so h_new[j] is ready while the PE still works on
                # later chunks (keeps the PE dense across step boundaries
                # and HAM at K=8/8).  The onehot-augmented matmul goes
                # first in each accumulation group: it does not depend on
                # h, so next-step PE work is issueable during the gate
                # latency window.
                hn = []
                for j in range(HC):
                    # onehot-augmented matmuls first (h-independent; K=8 on
                    # row strips 0/32/64 -> the three run concurrently)
                    pss = {}
                    for s, part in enumerate((j, HC + j, 2 * HC + j)):
                        ps = gpsum.tile([128, B], F32, tag="g", bufs=5,
                                        name=f"g{m}_{part}")
                        nc.tensor.matmul(
                            ps[:], gaug[32 * s:32 * s + 8, bass.ts(part, 128)],
                            onehot[32 * s:32 * s + 8, m:m + B],
                            start=True, stop=False, tile_position=(32 * s, 0),
                        )
                        pss[part] = ps
                    for part in (j, HC + j, 2 * HC + j):
                        ps = pss[part]
                        for k in range(HC):
                            nc.tensor.matmul(
                                ps[:], wtj[part][:, k, :], h[k][:],
                                start=False, stop=(k == HC - 1),
                            )
                    r = gates.tile([128, B], BF16, tag="r", bufs=6, name=f"r{m}_{j}")
                    nc.scalar.activation(r[:], pss[j][:], AF.Sigmoid)
                    z = gates.tile([128, B], BF16, tag="z", bufs=6, name=f"z{m}_{j}")
                    nc.scalar.activation(z[:], pss[HC + j][:], AF.Sigmoid)
                    tmp = gates.tile([128, B], BF16, tag="tmp", bufs=6, name=f"t{m}_{j}")
                    nc.vector.tensor_mul(tmp[:], r[:], pss[2 * HC + j][:])
                    tmp2 = gates.tile([128, B], BF16, tag="tmp2", bufs=6, name=f"u{m}_{j}")
                    nc.vector.tensor_add(tmp2[:], tmp[:], gin[:, j, m:m + B])
                    cand = gates.tile([128, B], BF16, tag="cand", bufs=6, name=f"c{m}_{j}")
                    nc.scalar.activation(cand[:], tmp2[:], AF.Tanh)
                    ve = nc.vector
                    d = gates.tile([128, B], BF16, tag="d", bufs=6, name=f"d{m}_{j}")
                    ve.tensor_sub(d[:], h[j][:], cand[:])
                    e = gates.tile([128, B], BF16, tag="e", bufs=6, name=f"e{m}_{j}")
                    ve.tensor_mul(e[:], z[:], d[:])
                    ht = hpool.tile([128, B], BF16, tag="h", bufs=12, name=f"h{m}_{j}")
                    ve.tensor_add(ht[:], cand[:], e[:])
                    hn.append(ht)
                h = hn
                if phase == "gru":
                    continue

                # classifier for step m: q-part then pos/neg heads
                qp = spsum.tile([32, B], F32, tag="s", bufs=2, name=f"q{m}")
                for k in range(HC):
                    nc.tensor.matmul(
                        qp[:], w1[:, HC + k, :], h[k][:],
                        start=(k == 0), stop=(k == HC - 1),
                    )
                for c, vp in ((0, v1pad), (1, n1pad)):
                    pre = cls.tile([32, B], BF16, tag="pre", bufs=4, name=f"pre{m}_{c}")
                    nc.vector.scalar_tensor_tensor(
                        out=pre[:], in0=qp[:], scalar=b1[:, 0:1],
                        in1=vp[:, m + 1:m + 1 + B],
                        op0=ALU.add, op1=ALU.add,
                    )
                    h1 = cls.tile([32, B], BF16, tag="h1", bufs=4, name=f"h1{m}_{c}")
                    nc.vector.tensor_scalar_max(h1[:], pre[:], 0.0)
                    for ch in range(BC):
                        nc.tensor.matmul(
                            logits[:, 2 * c + ch, m:m + 1],
                            h1[:, bass.ts(ch, 128)], w2[:, c:c + 1],
                            start=True, stop=True,
                        )

            # --- softplus + mask-weight reduce (single merged chain) --
            # logits already hold s0 = -+(h1 @ W2.T); s = s0 + (-+b2);
            # softplus(s) = max(s,0) + ln(1 + exp(-|s|))
            partials = cls.tile([128, 4, 1], F32, tag="part", name="partials")
            if phase == "gru":
                nc.scalar.activation(partials[:, :, 0], h[0][:, 0:4], AF.Copy)
            elif phase == "cls":
                nc.scalar.activation(partials[:, :, 0], logits[:, :, 0], AF.Copy)
            else:
                s = cls.tile([128, 4, K], F32, tag="s_aff", name="s_aff")
                nc.vector.tensor_add(s[:], logits[:], b2pat[:])
                rl = cls.tile([128, 4, K], F32, tag="s_rl", name="s_rl")
                nc.vector.tensor_scalar_max(rl[:], s[:], 0.0)
                nab = cls.tile([128, 4, K], F32, tag="s_nab", name="s_nab")
                # -|s| = s - 2*max(s,0)
                nc.vector.scalar_tensor_tensor(
                    out=nab[:], in0=rl[:], scalar=-2.0, in1=s[:],
                    op0=ALU.mult, op1=ALU.add,
                )
                ex = cls.tile([128, 4, K], F32, tag="s_ex", name="s_ex")
                nc.scalar.activation(ex[:], nab[:], AF.Exp)
                lg = cls.tile([128, 4, K], F32, tag="s_lg", name="s_lg")
                nc.scalar.activation(lg[:], ex[:], AF.Ln, bias=1.0)
                sp = cls.tile([128, 4, K], F32, tag="sp", name="sp")
                nc.vector.tensor_add(sp[:], rl[:], lg[:])
                # (tensor_tensor_reduce crashes this runtime; use mul+reduce)
                tr = cls.tile([128, 4, K], F32, tag="tr", name="tr")
                nc.vector.tensor_mul(tr[:], sp[:], wm[:])
                nc.vector.reduce_sum(partials[:], tr[:],
                                     axis=mybir.AxisListType.X)
            nc.sync.dma_start(d_out[:], partials[:, :, 0])

    nc.compile()
    _NC_CACHE["nc"] = nc
    return nc


def _threefry_pair(k0, k1, x0, x1):
    """numpy port of jax's threefry2x32 primitive (verified bit-exact)."""
    x0 = x0.astype(np.uint32).copy()
    x1 = x1.astype(np.uint32).copy()
    ks0 = np.uint32(k0)
    ks1 = np.uint32(k1)
    ks2 = np.uint32(ks0 ^ ks1 ^ np.uint32(0x1BD11BDA))

    def rotl(x, d):
        return ((x << np.uint32(d)) | (x >> np.uint32(32 - d))).astype(np.uint32)

    rots = [[13, 15, 26, 6], [17, 29, 16, 24]]
    x0 = (x0 + ks0).astype(np.uint32)
    x1 = (x1 + ks1).astype(np.uint32)
    ks = [ks1, ks2, ks0]
    for i in range(5):
        for r in rots[i % 2]:
            x0 = (x0 + x1).astype(np.uint32)
            x1 = np.uint32(rotl(x1, r) ^ x0)
        x0 = (x0 + ks[i % 3]).astype(np.uint32)
        x1 = (x1 + ks[(i + 1) % 3] + np.uint32(i + 1)).astype(np.uint32)
    return x0, x1


def _uniform_lt(key, shape, thresh):
    """jax.random.uniform(key, shape) < thresh, threefry-partitionable spec."""
    num = int(np.prod(shape))
    b1, b2 = _threefry_pair(key[0], key[1], np.zeros(num, np.uint32),
                            np.arange(num, dtype=np.uint32))
    bits = b1 ^ b2
    fl = ((bits >> np.uint32(9)) | np.uint32(0x3F800000)).view(np.float32) \
        - np.float32(1.0)
    fl = np.maximum(fl, np.float32(0.0))
    return (fl < np.float32(thresh)).reshape(shape)


def _sub_masks():
    """The reference's input-independent Bernoulli(P_SUB) masks
    (jax.random key(42) -> split -> uniform < P_SUB)."""
    if "subs" not in _NC_CACHE:
        b1, b2 = _threefry_pair(0, 42, np.zeros(2, np.uint32),
                                np.arange(2, dtype=np.uint32))
        sub_p = _uniform_lt((b1[0], b2[0]), (T, K, N), P_SUB)
        sub_n = _uniform_lt((b1[1], b2[1]), (T, K, N), P_SUB)
        _NC_CACHE["subs"] = (sub_p, sub_n)
    return _NC_CACHE["subs"]


def _bf16(x):
    return np.ascontiguousarray(np.asarray(x, dtype=np.float32)).astype(
        ml_dtypes.bfloat16
    )


def build_in_maps(inputs):
    """Host-side prep: returns (in_maps, cnt_p, cnt_n)."""
    return _prep(**{k: v for k, v in inputs.items() if k not in ("t", "n")})


def _prep(vision, belief_features, actions, env_zeros, negative_inds,
          emb, W_ih, W_hh, b_ih, b_hh, W1, b1, W2, b2, **_unused):
    vision = np.asarray(vision, np.float32)
    belief = np.asarray(belief_features, np.float32)
    actions = np.asarray(actions, np.int64)
    env_zeros = np.asarray(env_zeros, np.int64)
    negative_inds = np.asarray(negative_inds, np.int64)
    emb = np.asarray(emb, np.float32)
    W_ih = np.asarray(W_ih, np.float32)
    W_hh = np.asarray(W_hh, np.float32)
    b_ih = np.asarray(b_ih, np.float32)
    b_hh = np.asarray(b_hh, np.float32)
    W1 = np.asarray(W1, np.float32)
    b1v = np.asarray(b1, np.float32)
    W2 = np.asarray(W2, np.float32)
    b2v = np.asarray(b2, np.float32)

    # ---- host-side parameter folding (O(params) only) -----------------
    # G8[a] = x_a @ W_ih.T + b_ih for the 7 actions + zero pad (row 7)
    G8 = np.concatenate([emb, np.zeros((1, A), np.float32)], 0) @ W_ih.T + b_ih
    gaug8 = np.empty((8, G), np.float32)
    gaug8[:, :2 * H] = G8[:, :2 * H] + b_hh[None, :2 * H]    # r,z: gi + b_hh
    gaug8[:, 2 * H:] = np.tile(b_hh[None, 2 * H:], (8, 1))   # n:   b_hh only
    # replicate at partition strips 0/32/64 for row-tiled concurrent matmuls
    gaug = np.zeros((96, G), np.float32)
    for s in range(3):
        gaug[32 * s:32 * s + 8] = gaug8
    wt = np.ascontiguousarray(
        W_hh.T.reshape(HC, 128, G).transpose(1, 0, 2))       # [128, HC, G]
    w1abt = np.ascontiguousarray(
        W1.T.reshape(8, 128, 32).transpose(1, 0, 2))         # [128, 8, 32]
    w2sgn = np.stack([-W2[0], W2[0]], axis=1)                # [32, 2]
    b2f = float(b2v.reshape(-1)[0])
    b2pat = np.empty((128, 4, K), np.float32)
    b2pat[:, 0:2, :] = -b2f
    b2pat[:, 2:4, :] = b2f

    # ---- masks (host): valid & subsample, weighted --------------------
    sub_p, sub_n = _sub_masks()
    r = np.arange(T + K)[:, None, None]
    c = np.arange(K)[None, :, None]
    z = env_zeros[None, None, :, :]
    zero_hit = np.any((z >= (r - c + 1)[..., None]) & (z <= (r + 1)[..., None]),
                      axis=-1)
    valid_full = (r >= c) & (r < T - 1) & (~zero_hit)        # (T+K, K, N)
    idx = np.arange(T)[:, None] + np.arange(K)[None, :]
    valid = valid_full[idx, np.arange(K)[None, :]]           # (T, K, N)
    mask_p = valid & sub_p
    mask_n = valid & sub_n
    wmask_p = WEIGHT[None, :, None] * mask_p                 # (T, K, N) f32
    wmask_n = WEIGHT[None, :, None] * mask_n
    cnt_p = float(mask_p.sum())
    cnt_n = float(mask_n.sum())

    # ---- per-core inputs ----------------------------------------------
    negatives = vision.reshape(T * N, H)[negative_inds].reshape(T, N, H)

    def chunkT(x):  # (T=B, H) -> [128, HC, B] feature-major chunks
        return np.ascontiguousarray(x.T.reshape(HC, 128, B).transpose(1, 0, 2))

    in_maps = []
    for e in range(NCORES):
        a_pad = np.concatenate([actions[:, e], np.full(K - 1, 7, np.int64)])
        onehot = np.zeros((96, PADW), np.float32)
        for s in range(3):
            onehot[32 * s + a_pad, np.arange(PADW)] = 1.0
        gin = np.ascontiguousarray(
            G8[a_pad][:, 2 * H:].T.reshape(HC, 128, PADW).transpose(1, 0, 2))
        in_maps.append({
            "wt": _bf16(wt),
            "gaug": _bf16(gaug),
            "onehot": _bf16(onehot),
            "gin": _bf16(gin),
            "belief": _bf16(chunkT(belief[:, e, :])),
            "vis": _bf16(chunkT(vision[:, e, :])),
            "neg": _bf16(chunkT(negatives[:, e, :])),
            "w1abt": _bf16(w1abt),
            "w2sgn": _bf16(w2sgn),
            "b1": np.ascontiguousarray(b1v.reshape(32, 1)),
            "b2pat": b2pat,
            "wmask": np.ascontiguousarray(np.concatenate(
                [wmask_p[:, :, e].reshape(BC, 128, K),
                 wmask_n[:, :, e].reshape(BC, 128, K)],
                axis=0).transpose(1, 0, 2)),
        })

    return in_maps, cnt_p, cnt_n


def kernel(**inputs):
    in_maps, cnt_p, cnt_n = build_in_maps(inputs)
    nc = _build_bass()
    res = run_bass_kernel_spmd(nc, in_maps, core_ids=list(range(NCORES)))
    parts = np.stack([res.results[i]["partials"] for i in range(NCORES)])
    sp_num = float(parts[:, :, 0:2].sum(dtype=np.float64))
    sn_num = float(parts[:, :, 2:4].sum(dtype=np.float64))
    loss = (sp_num / max(cnt_p, 1.0) + sn_num / max(cnt_n, 1.0)) * LOSS_FACTOR
    return np.float32(loss)
